# revision 12
# baseline (speedup 1.0000x reference)
"""DeepseekV3 MoE "calibrate-all-experts" kernel for 8 Trainium2 NeuronCores.

Key observation: the reference multiplies every non-selected expert's
output by a dense weight of exactly 0, so only the top-8 experts per
token contribute to the result. The device therefore only computes the
routed experts on their *selected* tokens (4x less matmul work than the
dense formulation).

Split of work:
  host   - router (sigmoid scores -> top-8 -> renormalized weights),
           token gather/permutation, final scatter-add combine.
  device - per-core: 4 routed experts' FFNs (bf16 matmuls, fp32 PSUM)
           on gathered token columns, with the per-token routing weight
           folded into the output evacuation; plus a 1/8 slice (along
           the intermediate dim) of the shared expert over all tokens.

Experts are assigned to (core, slot) by sorting token counts so that
every slot has a uniform capacity across cores (SPMD: one program for
all 8 cores). Cores exchange nothing on device; the host adds the 8
shared-expert partials and scatter-adds the per-expert outputs.
"""
import sys

if '/opt/trn_rl_repo' not in sys.path:
    sys.path.insert(0, '/opt/trn_rl_repo')

import numpy as np
import ml_dtypes

import concourse.bass as bass
import concourse.mybir as mybir
import concourse.tile as tile
from concourse import bacc
from concourse.bass import ds, ts
from concourse.bass_utils import run_bass_kernel_spmd

F32 = mybir.dt.float32
BF16 = mybir.dt.bfloat16
P = 128
BF = ml_dtypes.bfloat16

TOP_K = 8
ROUTED_SCALING = 2.5
N_CORES = 8

# test harness hooks (ignored in grading)
_RUN_KW = {}
_LAST_RES = None


def _blocks(c):
    """Split a capacity into column blocks of <=512 (PSUM bank limit)."""
    out, off = [], 0
    while off < c:
        w = min(512, c - off)
        out.append((off, w))
        off += w
    return out


def build_sparse_nc(caps, T, H, I, IS, n_cores):
    """One SPMD program: len(caps) routed expert slots + shared slice."""
    E_LOC = len(caps)
    C_TOT = sum(caps)
    KH = H // P           # contraction tiles over H
    KI = I // P           # expert intermediate planes
    IS_LOC = IS // n_cores
    KIS = IS_LOC // P     # shared intermediate planes per core
    TH = T // 2           # shared-expert token half
    assert H % P == 0 and I % P == 0 and IS_LOC % P == 0 and T % 1024 == 0

    nc = bacc.Bacc("TRN2", target_bir_lowering=False, debug=False,
                   num_devices=n_cores)

    xg = nc.dram_tensor("xg", [H, C_TOT], BF16, kind="ExternalInput")
    wrow = nc.dram_tensor("wrow", [C_TOT], BF16, kind="ExternalInput")
    wg = nc.dram_tensor("wg", [E_LOC, H, I], BF16, kind="ExternalInput")
    wu = nc.dram_tensor("wu", [E_LOC, H, I], BF16, kind="ExternalInput")
    wd = nc.dram_tensor("wd", [E_LOC, I, H], BF16, kind="ExternalInput")
    wsg = nc.dram_tensor("wsg", [H, IS_LOC], BF16, kind="ExternalInput")
    wsu = nc.dram_tensor("wsu", [H, IS_LOC], BF16, kind="ExternalInput")
    wsd = nc.dram_tensor("wsd", [IS_LOC, H], BF16, kind="ExternalInput")
    xT = nc.dram_tensor("xT", [H, T], BF16, kind="ExternalInput")
    yT = nc.dram_tensor("yT", [H, C_TOT], BF16, kind="ExternalOutput")
    pT = nc.dram_tensor("pT", [H, T], BF16, kind="ExternalOutput")

    xg_t = xg.ap().rearrange("(ko p) c -> p ko c", p=P)
    xT_t = xT.ap().rearrange("(ko p) t -> p ko t", p=P)

    SILU = mybir.ActivationFunctionType.Silu

    H2 = H // 2
    with tile.TileContext(nc) as tc:
        with (
            tc.tile_pool(name="xpool", bufs=2) as xpool,      # xg / xT halves
            tc.tile_pool(name="wpool", bufs=2) as wpool,      # gate/up weights
            tc.tile_pool(name="wdpool", bufs=3) as wdpool,    # down wt halves
            tc.tile_pool(name="wrpool", bufs=2) as wrpool,    # wrow bcast
            tc.tile_pool(name="spool", bufs=2) as spool,      # silu(g) fp32
            tc.tile_pool(name="apool", bufs=2) as apool,      # activations
            tc.tile_pool(name="ypool", bufs=3) as ypool,      # out staging
            tc.tile_pool(name="pp", bufs=8, space="PSUM") as pp,
        ):
            xes = {}
            wrs = {}
            wgs = {}
            wus = {}
            wds = {}

            def fetch_x(j):
                c_off = sum(caps[:j])
                C = caps[j]
                xe = xpool.tile([P, KH, C], BF16, tag="x", name=f"xe{j}")
                nc.sync.dma_start(xe[:], xg_t[:, :, ds(c_off, C)])
                xes[j] = xe
                wr = wrpool.tile([P, C], BF16, tag="wr", name=f"wr{j}")
                nc.sync.dma_start(
                    wr[:], wrow.ap()[ds(c_off, C)].partition_broadcast(P))
                wrs[j] = wr

            def fetch_wg(j):
                w_sb = wpool.tile([P, KH, I], BF16, tag="w", name=f"wg{j}")
                nc.sync.dma_start(
                    w_sb[:], wg.ap()[j].rearrange("(ko p) i -> p ko i", p=P))
                wgs[j] = w_sb

            def fetch_wu(j):
                w_sb = wpool.tile([P, KH, I], BF16, tag="w", name=f"wu{j}")
                nc.sync.dma_start(
                    w_sb[:], wu.ap()[j].rearrange("(ko p) i -> p ko i", p=P))
                wus[j] = w_sb

            def fetch_wd(j, h):
                w_sb = wdpool.tile([P, KI, H2], BF16, tag="wd",
                                   name=f"wd{j}_{h}")
                nc.sync.dma_start(
                    w_sb[:], wd.ap()[j][:, ds(h * H2, H2)].rearrange(
                        "(ip p) h -> p ip h", p=P))
                wds[(j, h)] = w_sb

            fetch_x(0)
            fetch_wg(0)
            fetch_wu(0)
            fetch_wd(0, 0)
            fetch_wd(0, 1)

            for j in range(E_LOC):
                C = caps[j]
                c_off = sum(caps[:j])
                blks = _blocks(C)
                xe, wr = xes.pop(j), wrs.pop(j)
                wg_sb, wu_sb = wgs.pop(j), wus.pop(j)

                a_sb = apool.tile([P, KI, C], BF16, tag="a", name=f"a{j}")
                # ---- gate projection (all i-tiles) ----
                sgs = []
                for it in range(KI):
                    pgs = [pp.tile([P, bw], F32, tag="ps", name=f"pg{j}_{it}_{bi}")
                           for bi, (bo, bw) in enumerate(blks)]
                    for k in range(KH):
                        lhs = wg_sb[:, k, ts(it, P)]
                        for (bo, bw), pg in zip(blks, pgs):
                            nc.tensor.matmul(pg[:], lhs, xe[:, k, ds(bo, bw)],
                                             start=(k == 0), stop=(k == KH - 1))
                    sg = spool.tile([P, C], F32, tag="sg", name=f"sg{j}_{it}")
                    for (bo, bw), pg in zip(blks, pgs):
                        nc.scalar.activation(sg[:, ds(bo, bw)], pg[:], SILU)
                    sgs.append(sg)
                    # ---- up projection ----
                    pus = [pp.tile([P, bw], F32, tag="ps", name=f"pu{j}_{it}_{bi}")
                           for bi, (bo, bw) in enumerate(blks)]
                    for k in range(KH):
                        lhs = wu_sb[:, k, ts(it, P)]
                        for (bo, bw), pu in zip(blks, pus):
                            nc.tensor.matmul(pu[:], lhs, xe[:, k, ds(bo, bw)],
                                             start=(k == 0), stop=(k == KH - 1))
                    for (bo, bw), pu in zip(blks, pus):
                        nc.vector.tensor_mul(a_sb[:, it, ds(bo, bw)],
                                             sgs[it][:, ds(bo, bw)], pu[:])
                    if it == 0:
                        if j + 1 < E_LOC:
                            fetch_x(j + 1)   # slot already free: fires now
                            fetch_wg(j + 1)  # trigger waits gate(j) end
                        else:
                            w_sb = wpool.tile([P, KH, IS_LOC], BF16, tag="w",
                                              name="wsg_sb")
                            nc.sync.dma_start(
                                w_sb[:],
                                wsg.ap().rearrange("(ko p) i -> p ko i", p=P))
                            wgs["s"] = w_sb
                    if it == KI - 1:
                        if j + 1 < E_LOC:
                            fetch_wu(j + 1)  # trigger waits up(j) end
                        else:
                            w_sb = wpool.tile([P, KH, IS_LOC], BF16, tag="w",
                                              name="wsu_sb")
                            nc.sync.dma_start(
                                w_sb[:],
                                wsu.ap().rearrange("(ko p) i -> p ko i", p=P))
                            wus["s"] = w_sb

                # ---- down projection: out[h_tile, c] ----
                for ht in range(KH):
                    half = ht // (KH // 2)
                    wd_sb = wds[(j, half)]
                    pos = [pp.tile([P, bw], F32, tag="ps", name=f"po{j}_{ht}_{bi}")
                           for bi, (bo, bw) in enumerate(blks)]
                    for ip in range(KI):
                        lhs = wd_sb[:, ip, ts(ht - half * (KH // 2), P)]
                        for (bo, bw), po in zip(blks, pos):
                            nc.tensor.matmul(po[:], lhs, a_sb[:, ip, ds(bo, bw)],
                                             start=(ip == 0), stop=(ip == KI - 1))
                    yt = ypool.tile([P, C], BF16, tag="y", name=f"y{j}_{ht}")
                    for (bo, bw), po in zip(blks, pos):
                        nc.vector.tensor_mul(yt[:, ds(bo, bw)], po[:],
                                             wr[:, ds(bo, bw)])
                    nc.sync.dma_start(yT.ap()[ds(ht * P, P), ds(c_off, C)],
                                      yt[:])
                    if ht == 0 and j + 1 < E_LOC:
                        fetch_wd(j + 1, 0)
                    if ht == KH // 2 and j + 1 < E_LOC:
                        fetch_wd(j + 1, 1)
                    if ht == 0 and j + 1 == E_LOC:
                        # shared-expert down weights [P, KIS, H] fit a slot
                        w_sb = wdpool.tile([P, KIS, H], BF16, tag="wd",
                                           name="wsd_sb")
                        nc.sync.dma_start(
                            w_sb[:],
                            wsd.ap().rearrange("(ip p) h -> p ip h", p=P))
                        wds["s"] = w_sb
                    if ht == KH // 2 and j + 1 == E_LOC:
                        for s2 in range(2):
                            xh = xpool.tile([P, KH, TH], BF16, tag="x",
                                            name=f"xh{s2}")
                            nc.sync.dma_start(
                                xh[:], xT_t[:, :, ds(s2 * TH, TH)])
                            xes[f"s{s2}"] = xh
                wds.pop((j, 0))
                wds.pop((j, 1))

            # ---------------- shared expert (IS-sharded slice) -----------
            wsg_sb = wgs.pop("s")
            wsu_sb = wus.pop("s")
            wsd_sb = wds.pop("s")

            sblk = _blocks(TH)
            for t2 in range(2):
                xh = xes.pop(f"s{t2}")
                a_sh = apool.tile([P, KIS, TH], BF16, tag="a", name=f"ash{t2}")
                for it in range(KIS):
                    pgs = [pp.tile([P, bw], F32, tag="ps", name=f"spg{t2}_{it}_{bi}")
                           for bi, (bo, bw) in enumerate(sblk)]
                    for k in range(KH):
                        lhs = wsg_sb[:, k, ts(it, P)]
                        for (bo, bw), pg in zip(sblk, pgs):
                            nc.tensor.matmul(pg[:], lhs, xh[:, k, ds(bo, bw)],
                                             start=(k == 0), stop=(k == KH - 1))
                    sgb = [spool.tile([P, bw], F32, tag="sg",
                                      name=f"ssg{t2}_{it}_{bi}")
                           for bi, (bo, bw) in enumerate(sblk)]
                    for (bo, bw), pg, sg in zip(sblk, pgs, sgb):
                        nc.scalar.activation(sg[:], pg[:], SILU)
                    pus = [pp.tile([P, bw], F32, tag="ps", name=f"spu{t2}_{it}_{bi}")
                           for bi, (bo, bw) in enumerate(sblk)]
                    for k in range(KH):
                        lhs = wsu_sb[:, k, ts(it, P)]
                        for (bo, bw), pu in zip(sblk, pus):
                            nc.tensor.matmul(pu[:], lhs, xh[:, k, ds(bo, bw)],
                                             start=(k == 0), stop=(k == KH - 1))
                    for (bo, bw), pu, sg in zip(sblk, pus, sgb):
                        nc.vector.tensor_mul(a_sh[:, it, ds(bo, bw)],
                                             sg[:], pu[:])

                for ht in range(KH):
                    pos = [pp.tile([P, bw], F32, tag="ps", name=f"spo{t2}_{ht}_{bi}")
                           for bi, (bo, bw) in enumerate(sblk)]
                    for ip in range(KIS):
                        lhs = wsd_sb[:, ip, ts(ht, P)]
                        for (bo, bw), po in zip(sblk, pos):
                            nc.tensor.matmul(po[:], lhs, a_sh[:, ip, ds(bo, bw)],
                                             start=(ip == 0),
                                             stop=(ip == KIS - 1))
                    for bi, ((bo, bw), po) in enumerate(zip(sblk, pos)):
                        pb = ypool.tile([P, bw], BF16, tag="y",
                                        name=f"pb{t2}_{ht}_{bi}")
                        nc.vector.tensor_copy(pb[:], po[:])
                        nc.sync.dma_start(
                            pT.ap()[ds(ht * P, P), ds(t2 * TH + bo, bw)],
                            pb[:])

    nc.compile()
    return nc


_NC_CACHE = {}


def _get_nc(caps, T, H, I, IS, n_cores):
    key = (tuple(caps), T, H, I, IS, n_cores)
    if key not in _NC_CACHE:
        _NC_CACHE[key] = build_sparse_nc(list(caps), T, H, I, IS, n_cores)
    return _NC_CACHE[key]


def _route(x, gate_weight):
    """fp32 host router identical to the reference semantics."""
    logits = x @ np.asarray(gate_weight, np.float32).T
    scores = 1.0 / (1.0 + np.exp(-logits))
    topk_i = np.argpartition(-scores, TOP_K - 1, axis=1)[:, :TOP_K]
    topk_w = np.take_along_axis(scores, topk_i, axis=1)
    topk_w = topk_w / (topk_w.sum(1, keepdims=True) + 1e-20) * ROUTED_SCALING
    return topk_i, topk_w


def kernel(hidden_states, gate_weight, w_gate, w_up, w_down,
           ws_gate, ws_up, ws_down):
    global _LAST_RES
    B, S, H = hidden_states.shape
    T = B * S
    E, _, I = w_gate.shape
    IS = ws_gate.shape[1]
    n_cores = N_CORES
    E_LOC = E // n_cores
    IS_LOC = IS // n_cores

    x = np.asarray(hidden_states, np.float32).reshape(T, H)
    topk_i, topk_w = _route(x, gate_weight)

    # token lists per expert
    idx_by_e = []
    w_by_e = []
    for e in range(E):
        sel = (topk_i == e)
        rows = np.nonzero(sel.any(1))[0]
        idx_by_e.append(rows)
        w_by_e.append((topk_w[rows] * sel[rows]).sum(1).astype(np.float32))
    cnt = np.array([len(i) for i in idx_by_e])

    # slot assignment: sort experts by count desc, group into n_cores
    order = np.argsort(-cnt, kind="stable")
    caps = []
    slot_experts = []  # [slot][core] -> expert id
    for j in range(E_LOC):
        grp = order[j * n_cores:(j + 1) * n_cores]
        caps.append(max(64, int(-(-cnt[grp].max() // 8) * 8)))
        slot_experts.append(list(grp))
    C_TOT = int(sum(caps))

    xT = np.ascontiguousarray(x.T)
    xTb = xT.astype(BF)

    in_maps = []
    for c in range(n_cores):
        idx_cat = np.zeros(C_TOT, np.int64)
        wr_cat = np.zeros(C_TOT, np.float32)
        off = 0
        exps = []
        for j in range(E_LOC):
            e = slot_experts[j][c]
            exps.append(e)
            n = cnt[e]
            idx_cat[off:off + n] = idx_by_e[e]
            wr_cat[off:off + n] = w_by_e[e]
            off += caps[j]
        in_maps.append({
            "xg": np.ascontiguousarray(xTb[:, idx_cat]),
            "wrow": wr_cat.astype(BF),
            "wg": np.ascontiguousarray(w_gate[exps]).astype(BF),
            "wu": np.ascontiguousarray(w_up[exps]).astype(BF),
            "wd": np.ascontiguousarray(w_down[exps]).astype(BF),
            "wsg": np.ascontiguousarray(
                ws_gate[:, c * IS_LOC:(c + 1) * IS_LOC]).astype(BF),
            "wsu": np.ascontiguousarray(
                ws_up[:, c * IS_LOC:(c + 1) * IS_LOC]).astype(BF),
            "wsd": np.ascontiguousarray(
                ws_down[c * IS_LOC:(c + 1) * IS_LOC, :]).astype(BF),
            "xT": xTb,
        })

    nc = _get_nc(caps, T, H, I, IS, n_cores)
    res = run_bass_kernel_spmd(nc, in_maps, core_ids=list(range(n_cores)),
                               **_RUN_KW)
    _LAST_RES = res

    # host combine: scatter-add routed outputs + sum shared partials
    out = np.zeros((T, H), np.float32)
    for c in range(n_cores):
        y = res.results[c]["yT"].T.astype(np.float32)  # [C_TOT, H]
        off = 0
        for j in range(E_LOC):
            e = slot_experts[j][c]
            n = cnt[e]
            out[idx_by_e[e]] += y[off:off + n]
            off += caps[j]
        out += res.results[c]["pT"].T.astype(np.float32)

    return np.ascontiguousarray(
        out.reshape(B, S, H).astype(np.asarray(hidden_states).dtype))


# revision 13
# speedup vs baseline: 1.0578x; 1.0578x over previous
"""DeepseekV3 MoE "calibrate-all-experts" kernel for 8 Trainium2 NeuronCores.

Key observation: the reference multiplies every non-selected expert's
output by a dense weight of exactly 0, so only the top-8 experts per
token contribute to the result. The device therefore only computes the
routed experts on their *selected* tokens (4x less matmul work than the
dense formulation).

Split of work:
  host   - router (sigmoid scores -> top-8 -> renormalized weights),
           token gather/permutation, final scatter-add combine.
  device - per-core: 4 routed experts' FFNs (bf16 matmuls, fp32 PSUM)
           on gathered token columns, with the per-token routing weight
           folded into the output evacuation; plus a 1/8 slice (along
           the intermediate dim) of the shared expert over all tokens.

Experts are assigned to (core, slot) by sorting token counts so that
every slot has a uniform capacity across cores (SPMD: one program for
all 8 cores). Cores exchange nothing on device; the host adds the 8
shared-expert partials and scatter-adds the per-expert outputs.
"""
import sys

if '/opt/trn_rl_repo' not in sys.path:
    sys.path.insert(0, '/opt/trn_rl_repo')

import numpy as np
import ml_dtypes

import concourse.bass as bass
import concourse.mybir as mybir
import concourse.tile as tile
from concourse import bacc
from concourse.bass import ds, ts
from concourse.bass_utils import run_bass_kernel_spmd

F32 = mybir.dt.float32
BF16 = mybir.dt.bfloat16
P = 128
BF = ml_dtypes.bfloat16

TOP_K = 8
ROUTED_SCALING = 2.5
N_CORES = 8

# test harness hooks (ignored in grading)
_RUN_KW = {}
_LAST_RES = None


def _blocks(c):
    """Split a capacity into column blocks of <=512 (PSUM bank limit)."""
    out, off = [], 0
    while off < c:
        w = min(512, c - off)
        out.append((off, w))
        off += w
    return out


def build_sparse_nc(caps, T, H, I, IS, n_cores):
    """One SPMD program: len(caps) routed expert slots + shared slice."""
    E_LOC = len(caps)
    C_TOT = sum(caps)
    KH = H // P           # contraction tiles over H
    KI = I // P           # expert intermediate planes
    IS_LOC = IS // n_cores
    KIS = IS_LOC // P     # shared intermediate planes per core
    TH = T // 2           # shared-expert token half
    assert H % P == 0 and I % P == 0 and IS_LOC % P == 0 and T % 1024 == 0

    nc = bacc.Bacc("TRN2", target_bir_lowering=False, debug=False,
                   num_devices=n_cores)

    xg = nc.dram_tensor("xg", [H, C_TOT], BF16, kind="ExternalInput")
    wrow = nc.dram_tensor("wrow", [C_TOT], BF16, kind="ExternalInput")
    wg = nc.dram_tensor("wg", [E_LOC, H, I], BF16, kind="ExternalInput")
    wu = nc.dram_tensor("wu", [E_LOC, H, I], BF16, kind="ExternalInput")
    wd = nc.dram_tensor("wd", [E_LOC, I, H], BF16, kind="ExternalInput")
    wsg = nc.dram_tensor("wsg", [H, IS_LOC], BF16, kind="ExternalInput")
    wsu = nc.dram_tensor("wsu", [H, IS_LOC], BF16, kind="ExternalInput")
    wsd = nc.dram_tensor("wsd", [IS_LOC, H], BF16, kind="ExternalInput")
    xT = nc.dram_tensor("xT", [H, T], BF16, kind="ExternalInput")
    yT = nc.dram_tensor("yT", [H, C_TOT], BF16, kind="ExternalOutput")
    pT = nc.dram_tensor("pT", [H, T], BF16, kind="ExternalOutput")

    xg_t = xg.ap().rearrange("(ko p) c -> p ko c", p=P)
    xT_t = xT.ap().rearrange("(ko p) t -> p ko t", p=P)

    SILU = mybir.ActivationFunctionType.Silu

    H2 = H // 2
    with tile.TileContext(nc) as tc:
        with (
            tc.tile_pool(name="xpool", bufs=2) as xpool,      # xg / xT halves
            tc.tile_pool(name="wpool", bufs=2) as wpool,      # gate/up weights
            tc.tile_pool(name="wdpool", bufs=3) as wdpool,    # down wt halves
            tc.tile_pool(name="wrpool", bufs=2) as wrpool,    # wrow bcast
            tc.tile_pool(name="spool", bufs=2) as spool,      # silu(g) fp32
            tc.tile_pool(name="apool", bufs=2) as apool,      # activations
            tc.tile_pool(name="ypool", bufs=3) as ypool,      # out staging
            tc.tile_pool(name="pp", bufs=8, space="PSUM") as pp,
        ):
            xes = {}
            wrs = {}
            wgs = {}
            wus = {}
            wds = {}

            def fetch_x(j):
                c_off = sum(caps[:j])
                C = caps[j]
                xe = xpool.tile([P, KH, C], BF16, tag="x", name=f"xe{j}")
                nc.sync.dma_start(xe[:], xg_t[:, :, ds(c_off, C)])
                xes[j] = xe
                wr = wrpool.tile([P, C], BF16, tag="wr", name=f"wr{j}")
                nc.scalar.dma_start(
                    wr[:], wrow.ap()[ds(c_off, C)].partition_broadcast(P))
                wrs[j] = wr

            def fetch_wg(j):
                w_sb = wpool.tile([P, KH, I], BF16, tag="w", name=f"wg{j}")
                nc.sync.dma_start(
                    w_sb[:], wg.ap()[j].rearrange("(ko p) i -> p ko i", p=P))
                wgs[j] = w_sb

            def fetch_wu(j):
                w_sb = wpool.tile([P, KH, I], BF16, tag="w", name=f"wu{j}")
                nc.sync.dma_start(
                    w_sb[:], wu.ap()[j].rearrange("(ko p) i -> p ko i", p=P))
                wus[j] = w_sb

            def fetch_wd(j, h):
                w_sb = wdpool.tile([P, KI, H2], BF16, tag="wd",
                                   name=f"wd{j}_{h}")
                nc.sync.dma_start(
                    w_sb[:], wd.ap()[j][:, ds(h * H2, H2)].rearrange(
                        "(ip p) h -> p ip h", p=P))
                wds[(j, h)] = w_sb

            fetch_x(0)
            fetch_wg(0)
            fetch_wu(0)
            fetch_wd(0, 0)
            fetch_wd(0, 1)

            for j in range(E_LOC):
                C = caps[j]
                c_off = sum(caps[:j])
                blks = _blocks(C)
                xe, wr = xes.pop(j), wrs.pop(j)
                wg_sb, wu_sb = wgs.pop(j), wus.pop(j)

                a_sb = apool.tile([P, KI, C], BF16, tag="a", name=f"a{j}")
                # ---- gate projection (all i-tiles) ----
                sgs = []
                for it in range(KI):
                    pgs = [pp.tile([P, bw], F32, tag="ps", name=f"pg{j}_{it}_{bi}")
                           for bi, (bo, bw) in enumerate(blks)]
                    for k in range(KH):
                        lhs = wg_sb[:, k, ts(it, P)]
                        for (bo, bw), pg in zip(blks, pgs):
                            nc.tensor.matmul(pg[:], lhs, xe[:, k, ds(bo, bw)],
                                             start=(k == 0), stop=(k == KH - 1))
                    sg = spool.tile([P, C], F32, tag="sg", name=f"sg{j}_{it}")
                    for (bo, bw), pg in zip(blks, pgs):
                        nc.scalar.activation(sg[:, ds(bo, bw)], pg[:], SILU)
                    sgs.append(sg)
                    # ---- up projection ----
                    pus = [pp.tile([P, bw], F32, tag="ps", name=f"pu{j}_{it}_{bi}")
                           for bi, (bo, bw) in enumerate(blks)]
                    for k in range(KH):
                        lhs = wu_sb[:, k, ts(it, P)]
                        for (bo, bw), pu in zip(blks, pus):
                            nc.tensor.matmul(pu[:], lhs, xe[:, k, ds(bo, bw)],
                                             start=(k == 0), stop=(k == KH - 1))
                    for (bo, bw), pu in zip(blks, pus):
                        nc.vector.tensor_mul(a_sb[:, it, ds(bo, bw)],
                                             sgs[it][:, ds(bo, bw)], pu[:])
                    if it == 0:
                        if j + 1 < E_LOC:
                            fetch_x(j + 1)   # slot already free: fires now
                            fetch_wg(j + 1)  # trigger waits gate(j) end
                        else:
                            w_sb = wpool.tile([P, KH, IS_LOC], BF16, tag="w",
                                              name="wsg_sb")
                            nc.sync.dma_start(
                                w_sb[:],
                                wsg.ap().rearrange("(ko p) i -> p ko i", p=P))
                            wgs["s"] = w_sb
                    if it == KI - 1:
                        if j + 1 < E_LOC:
                            fetch_wu(j + 1)  # trigger waits up(j) end
                        else:
                            w_sb = wpool.tile([P, KH, IS_LOC], BF16, tag="w",
                                              name="wsu_sb")
                            nc.sync.dma_start(
                                w_sb[:],
                                wsu.ap().rearrange("(ko p) i -> p ko i", p=P))
                            wus["s"] = w_sb

                # ---- down projection: out[h_tile, c] ----
                for ht in range(KH):
                    half = ht // (KH // 2)
                    wd_sb = wds[(j, half)]
                    pos = [pp.tile([P, bw], F32, tag="ps", name=f"po{j}_{ht}_{bi}")
                           for bi, (bo, bw) in enumerate(blks)]
                    for ip in range(KI):
                        lhs = wd_sb[:, ip, ts(ht - half * (KH // 2), P)]
                        for (bo, bw), po in zip(blks, pos):
                            nc.tensor.matmul(po[:], lhs, a_sb[:, ip, ds(bo, bw)],
                                             start=(ip == 0), stop=(ip == KI - 1))
                    yt = ypool.tile([P, C], BF16, tag="y", name=f"y{j}_{ht}")
                    for (bo, bw), po in zip(blks, pos):
                        nc.vector.tensor_mul(yt[:, ds(bo, bw)], po[:],
                                             wr[:, ds(bo, bw)])
                    nc.gpsimd.dma_start(yT.ap()[ds(ht * P, P), ds(c_off, C)],
                                        yt[:])
                    if ht == 0 and j + 1 < E_LOC:
                        fetch_wd(j + 1, 0)
                    if ht == KH // 2 and j + 1 < E_LOC:
                        fetch_wd(j + 1, 1)
                    if ht == 0 and j + 1 == E_LOC:
                        # shared-expert down weights [P, KIS, H] fit a slot
                        w_sb = wdpool.tile([P, KIS, H], BF16, tag="wd",
                                           name="wsd_sb")
                        nc.sync.dma_start(
                            w_sb[:],
                            wsd.ap().rearrange("(ip p) h -> p ip h", p=P))
                        wds["s"] = w_sb
                    if ht == KH // 2 and j + 1 == E_LOC:
                        for s2 in range(2):
                            xh = xpool.tile([P, KH, TH], BF16, tag="x",
                                            name=f"xh{s2}")
                            nc.sync.dma_start(
                                xh[:], xT_t[:, :, ds(s2 * TH, TH)])
                            xes[f"s{s2}"] = xh
                wds.pop((j, 0))
                wds.pop((j, 1))

            # ---------------- shared expert (IS-sharded slice) -----------
            wsg_sb = wgs.pop("s")
            wsu_sb = wus.pop("s")
            wsd_sb = wds.pop("s")

            sblk = _blocks(TH)
            for t2 in range(2):
                xh = xes.pop(f"s{t2}")
                a_sh = apool.tile([P, KIS, TH], BF16, tag="a", name=f"ash{t2}")
                for it in range(KIS):
                    pgs = [pp.tile([P, bw], F32, tag="ps", name=f"spg{t2}_{it}_{bi}")
                           for bi, (bo, bw) in enumerate(sblk)]
                    for k in range(KH):
                        lhs = wsg_sb[:, k, ts(it, P)]
                        for (bo, bw), pg in zip(sblk, pgs):
                            nc.tensor.matmul(pg[:], lhs, xh[:, k, ds(bo, bw)],
                                             start=(k == 0), stop=(k == KH - 1))
                    sgb = [spool.tile([P, bw], F32, tag="sg",
                                      name=f"ssg{t2}_{it}_{bi}")
                           for bi, (bo, bw) in enumerate(sblk)]
                    for (bo, bw), pg, sg in zip(sblk, pgs, sgb):
                        nc.scalar.activation(sg[:], pg[:], SILU)
                    pus = [pp.tile([P, bw], F32, tag="ps", name=f"spu{t2}_{it}_{bi}")
                           for bi, (bo, bw) in enumerate(sblk)]
                    for k in range(KH):
                        lhs = wsu_sb[:, k, ts(it, P)]
                        for (bo, bw), pu in zip(sblk, pus):
                            nc.tensor.matmul(pu[:], lhs, xh[:, k, ds(bo, bw)],
                                             start=(k == 0), stop=(k == KH - 1))
                    for (bo, bw), pu, sg in zip(sblk, pus, sgb):
                        nc.vector.tensor_mul(a_sh[:, it, ds(bo, bw)],
                                             sg[:], pu[:])

                for ht in range(KH):
                    pos = [pp.tile([P, bw], F32, tag="ps", name=f"spo{t2}_{ht}_{bi}")
                           for bi, (bo, bw) in enumerate(sblk)]
                    for ip in range(KIS):
                        lhs = wsd_sb[:, ip, ts(ht, P)]
                        for (bo, bw), po in zip(sblk, pos):
                            nc.tensor.matmul(po[:], lhs, a_sh[:, ip, ds(bo, bw)],
                                             start=(ip == 0),
                                             stop=(ip == KIS - 1))
                    for bi, ((bo, bw), po) in enumerate(zip(sblk, pos)):
                        pb = ypool.tile([P, bw], BF16, tag="y",
                                        name=f"pb{t2}_{ht}_{bi}")
                        nc.vector.tensor_copy(pb[:], po[:])
                        nc.gpsimd.dma_start(
                            pT.ap()[ds(ht * P, P), ds(t2 * TH + bo, bw)],
                            pb[:])

    nc.compile()
    return nc


_NC_CACHE = {}


def _get_nc(caps, T, H, I, IS, n_cores):
    key = (tuple(caps), T, H, I, IS, n_cores)
    if key not in _NC_CACHE:
        _NC_CACHE[key] = build_sparse_nc(list(caps), T, H, I, IS, n_cores)
    return _NC_CACHE[key]


def _route(x, gate_weight):
    """fp32 host router identical to the reference semantics."""
    logits = x @ np.asarray(gate_weight, np.float32).T
    scores = 1.0 / (1.0 + np.exp(-logits))
    topk_i = np.argpartition(-scores, TOP_K - 1, axis=1)[:, :TOP_K]
    topk_w = np.take_along_axis(scores, topk_i, axis=1)
    topk_w = topk_w / (topk_w.sum(1, keepdims=True) + 1e-20) * ROUTED_SCALING
    return topk_i, topk_w


def kernel(hidden_states, gate_weight, w_gate, w_up, w_down,
           ws_gate, ws_up, ws_down):
    global _LAST_RES
    B, S, H = hidden_states.shape
    T = B * S
    E, _, I = w_gate.shape
    IS = ws_gate.shape[1]
    n_cores = N_CORES
    E_LOC = E // n_cores
    IS_LOC = IS // n_cores

    x = np.asarray(hidden_states, np.float32).reshape(T, H)
    topk_i, topk_w = _route(x, gate_weight)

    # token lists per expert
    idx_by_e = []
    w_by_e = []
    for e in range(E):
        sel = (topk_i == e)
        rows = np.nonzero(sel.any(1))[0]
        idx_by_e.append(rows)
        w_by_e.append((topk_w[rows] * sel[rows]).sum(1).astype(np.float32))
    cnt = np.array([len(i) for i in idx_by_e])

    # slot assignment: sort experts by count desc, group into n_cores
    order = np.argsort(-cnt, kind="stable")
    caps = []
    slot_experts = []  # [slot][core] -> expert id
    for j in range(E_LOC):
        grp = order[j * n_cores:(j + 1) * n_cores]
        caps.append(max(64, int(-(-cnt[grp].max() // 8) * 8)))
        slot_experts.append(list(grp))
    C_TOT = int(sum(caps))

    xT = np.ascontiguousarray(x.T)
    xTb = xT.astype(BF)

    in_maps = []
    for c in range(n_cores):
        idx_cat = np.zeros(C_TOT, np.int64)
        wr_cat = np.zeros(C_TOT, np.float32)
        off = 0
        exps = []
        for j in range(E_LOC):
            e = slot_experts[j][c]
            exps.append(e)
            n = cnt[e]
            idx_cat[off:off + n] = idx_by_e[e]
            wr_cat[off:off + n] = w_by_e[e]
            off += caps[j]
        in_maps.append({
            "xg": np.ascontiguousarray(xTb[:, idx_cat]),
            "wrow": wr_cat.astype(BF),
            "wg": np.ascontiguousarray(w_gate[exps]).astype(BF),
            "wu": np.ascontiguousarray(w_up[exps]).astype(BF),
            "wd": np.ascontiguousarray(w_down[exps]).astype(BF),
            "wsg": np.ascontiguousarray(
                ws_gate[:, c * IS_LOC:(c + 1) * IS_LOC]).astype(BF),
            "wsu": np.ascontiguousarray(
                ws_up[:, c * IS_LOC:(c + 1) * IS_LOC]).astype(BF),
            "wsd": np.ascontiguousarray(
                ws_down[c * IS_LOC:(c + 1) * IS_LOC, :]).astype(BF),
            "xT": xTb,
        })

    nc = _get_nc(caps, T, H, I, IS, n_cores)
    res = run_bass_kernel_spmd(nc, in_maps, core_ids=list(range(n_cores)),
                               **_RUN_KW)
    _LAST_RES = res

    # host combine: scatter-add routed outputs + sum shared partials
    out = np.zeros((T, H), np.float32)
    for c in range(n_cores):
        y = res.results[c]["yT"].T.astype(np.float32)  # [C_TOT, H]
        off = 0
        for j in range(E_LOC):
            e = slot_experts[j][c]
            n = cnt[e]
            out[idx_by_e[e]] += y[off:off + n]
            off += caps[j]
        out += res.results[c]["pT"].T.astype(np.float32)

    return np.ascontiguousarray(
        out.reshape(B, S, H).astype(np.asarray(hidden_states).dtype))
